# revision 1
# baseline (speedup 1.0000x reference)
"""Centroid-similarity (ProtoNet softmax) kernel for 8 trn2 NeuronCores.

Math (per reference):
    counts   = sum_n y[n, c]
    cent     = (y^T @ E) / max(counts, 1)          # divide_no_nan
    out      = softmax(-(|e|^2 + |c|^2 - 2 e.c), axis=C)
softmax is invariant to per-row constants, so |e|^2 drops out:
    out      = softmax(2*cross - sq_c), cross = E @ cent^T, sq_c = |cent|^2

Distribution: data-parallel over N. Each core gets an N/8 = 2048-row shard
of embeddings / y_true, computes partial (y^T E | counts) with the tensor
engine, AllReduces the [C, D+1] stats, then computes its own 2048 x C block
of logits + softmax. While the AllReduce runs, the PE transposes the local
E shard (needed because matmul #2 contracts over D, which must sit on the
partition axis).

Matmuls run as float32r (TF32) by default: 4x the fp32 matmul throughput;
emulated end-to-end softmax absmax error vs fp32 reference ~2e-3. The BIR
verifier requires every fp32r matmul operand to be produced by a rounding
compute op (DMA bit-copies don't count), so E/y are rounded into f32r tiles
by ACT/DVE copies that hide under the HBM load.
"""

import numpy as np

import concourse.bacc as bacc
import concourse.bass as bass
import concourse.mybir as mybir
import concourse.tile as tile
from concourse import masks
from concourse.bass_utils import run_bass_kernel_spmd
from concourse.tile import add_dep_helper

N, C, D = 16384, 128, 1024
CORES = 8
NS = N // CORES          # 2048 rows per core
P = 128                  # partition dim
NCH = NS // P            # 16 n-chunks per core
DCH = D // P             # 8 d-chunks
NB = NS // 512           # 4 moving-dim blocks for matmul #2

F32 = mybir.dt.float32
MM_DT = mybir.dt.float32r    # flip to F32 for exact-fp32 matmuls (4x slower)

AF = mybir.ActivationFunctionType
AX = mybir.AxisListType


def _build_kernel(tc: tile.TileContext, emb: bass.AP, yt: bass.AP, out: bass.AP,
                  stage: int = 7):
    nc = tc.nc

    with (
        tc.tile_pool(name="const", bufs=1) as const_pool,
        tc.tile_pool(name="persist", bufs=1) as persist,
        tc.tile_pool(name="echunks", bufs=4) as e_pool,
        tc.tile_pool(name="erchunks", bufs=NCH) as er_pool,
        tc.tile_pool(name="ychunks", bufs=4) as y_pool,
        tc.tile_pool(name="yrchunks", bufs=NCH) as yr_pool,
        tc.tile_pool(name="dram", bufs=1, space="DRAM") as dram_pool,
        tc.tile_pool(name="trps", bufs=4, space="PSUM") as tr_ps,
        tc.tile_pool(name="smalls", bufs=1) as smalls,
    ):
        ident = const_pool.tile([P, P], F32)
        masks.make_identity(nc, ident[:])
        ident_r = const_pool.tile([P, P], MM_DT)
        nc.scalar.copy(ident_r[:], ident[:])
        # counts matmul runs in bf16 (exact for one-hot y / ones; fp32r
        # can't write a 1-column PSUM destination)
        BF16 = mybir.dt.bfloat16
        ones_b = const_pool.tile([P, 1], BF16)
        nc.vector.memset(ones_b[:], 1.0)

        # ---- phase A: stream in shard; round to f32r; accumulate y^T E ----
        mm1_ctx = tc.tile_pool(name="mm1ps", bufs=1, space="PSUM")
        mm1_ps = mm1_ctx.__enter__()
        cent_ps = [mm1_ps.tile([P, 512], F32, name=f"cent_ps{h}") for h in range(2)]
        cnt_ps = mm1_ps.tile([P, 1], F32)
        er_tiles = []
        for i in range(NCH):
            y_t = y_pool.tile([P, C], F32, tag="y")
            e_t = e_pool.tile([P, D], F32, tag="e")
            nc.sync.dma_start(out=y_t[:], in_=yt[i * P:(i + 1) * P, :])
            nc.sync.dma_start(out=e_t[:], in_=emb[i * P:(i + 1) * P, :])
            # round fp32 -> tf32 tiles (alternate engines; hides under DMA)
            e_r = er_pool.tile([P, D], MM_DT, tag="er")
            y_r = yr_pool.tile([P, C], MM_DT, tag="yr")
            y_b = yr_pool.tile([P, C], BF16, tag="yb")
            if i % 2 == 0:
                nc.scalar.copy(e_r[:], e_t[:])
                nc.vector.tensor_copy(y_r[:], y_t[:])
                nc.vector.tensor_copy(y_b[:], y_t[:])
            else:
                nc.vector.tensor_copy(e_r[:], e_t[:])
                nc.scalar.copy(y_r[:], y_t[:])
                nc.scalar.copy(y_b[:], y_t[:])
            er_tiles.append(e_r)
            if stage < 2:
                continue
            first, last = (i == 0), (i == NCH - 1)
            for h in range(2):
                nc.tensor.matmul(
                    cent_ps[h][:],
                    lhsT=y_r[:],
                    rhs=e_r[:, h * 512:(h + 1) * 512],
                    start=first, stop=last,
                )
            mm1_last = nc.tensor.matmul(
                cnt_ps[:], lhsT=y_b[:], rhs=ones_b[:],
                start=first, stop=last,
            )

        if stage < 2:
            mm1_ctx.__exit__(None, None, None)
            return
        # ---- phase B: AllReduce the [C, D+1] stats across the 8 cores ----
        # high_priority: the stats -> AllReduce chain must win the per-engine
        # FIFO ordering races against the (long) phase-C transpose/copy work,
        # or the collective trigger slips ~40us down the ACT/gpsimd queues.
        stat_sb = persist.tile([P, D + 1], F32)
        ar_in = dram_pool.tile([P, D + 1], F32)
        ar_out = dram_pool.tile([P, D + 1], F32)
        gcnt = persist.tile([P, 1], F32)
        gcent = [persist.tile([P, P], F32, name=f"gcent{j}") for j in range(DCH)]
        with tc.high_priority():
            nc.scalar.copy(stat_sb[:, 0:512], cent_ps[0][:])
            nc.scalar.copy(stat_sb[:, 512:1024], cent_ps[1][:])
            nc.scalar.copy(stat_sb[:, D:D + 1], cnt_ps[:])
            mm1_ctx.__exit__(None, None, None)
            if stage < 3:
                return
            ar_dma = nc.sync.dma_start(out=ar_in[:], in_=stat_sb[:])
            nc.gpsimd.collective_compute(
                "AllReduce",
                mybir.AluOpType.add,
                replica_groups=[list(range(CORES))],
                ins=[ar_in.opt()],
                outs=[ar_out.opt()],
            )
            # counts column first (tiny) so the reciprocal chain starts
            # before the 512KB centroid payload finishes landing
            nc.sync.dma_start(out=gcnt[:], in_=ar_out[:, D:D + 1])
            for j in range(DCH):
                nc.sync.dma_start(out=gcent[j][:],
                                  in_=ar_out[:, j * P:(j + 1) * P])

        if stage < 4:
            return
        # ---- phase C: transpose E shard on PE while the AllReduce runs ----
        et = persist.tile([P, DCH * NS], MM_DT)  # d-chunk j at [:, j*NS:(j+1)*NS]
        k = 0
        for i in range(NCH):
            for j in range(DCH):
                tp = tr_ps.tile([P, P], F32, tag="tr")
                tr_inst = nc.tensor.transpose(
                    tp.bitcast(MM_DT),
                    er_tiles[i][:, j * P:(j + 1) * P], ident_r[:])
                # ordering-only edge (same PE queue): keep all transposes
                # after phase A so their ACT/DVE copies can't crowd out the
                # stats -> AllReduce chain in those engines' FIFOs
                add_dep_helper(tr_inst.ins, mm1_last.ins, sync=False,
                               reason="transposes after mm1")
                dst = et[:, j * NS + i * P: j * NS + (i + 1) * P]
                if k % 2 == 0:
                    cp_inst = nc.scalar.copy(dst, tp[:])
                else:
                    cp_inst = nc.vector.tensor_copy(dst, tp[:])
                # pin every ET copy after the AllReduce-input DMA so the
                # stats chain always wins the ACT/DVE FIFO placement races
                add_dep_helper(cp_inst.ins, ar_dma.ins, sync=False,
                               reason="et copies after ar_in dma")
                k += 1

        if stage < 5:
            return
        # ---- phase C2: cent2 = 2*cent, -sq_c, cent2^T — pipelined per d-chunk
        # so matmul #2's first accumulation starts as soon as chunk 0 is ready
        safe = smalls.tile([P, 1], F32)
        nc.vector.tensor_scalar_max(safe[:], gcnt[:], 1.0)
        r2 = smalls.tile([P, 1], F32)
        nc.vector.reciprocal(r2[:], safe[:])
        nc.vector.tensor_scalar_mul(r2[:], r2[:], 2.0)
        if stage < 5.4:
            return
        sq_tmp = persist.tile([P, D], F32)
        negsq = smalls.tile([P, 1], F32)
        cent2 = [persist.tile([P, P], MM_DT, name=f"cent2_{j}") for j in range(DCH)]
        centT = [persist.tile([P, C], MM_DT, name=f"centT{j}") for j in range(DCH)]
        for j in range(DCH):
            nc.vector.tensor_scalar_mul(cent2[j][:], gcent[j][:], r2[:, 0:1])
            tp = tr_ps.tile([P, P], F32, tag="tr")
            nc.tensor.transpose(tp.bitcast(MM_DT), cent2[j][:], ident_r[:])
            nc.scalar.copy(centT[j][:], tp[:])
            # negsq contribution off the critical path (only exp needs it)
            nc.scalar.square(sq_tmp[:, j * P:(j + 1) * P], cent2[j].bitcast(F32))
        nc.vector.reduce_sum(out=negsq[:], in_=sq_tmp[:], axis=AX.X)
        nc.vector.tensor_scalar_mul(negsq[:], negsq[:], -0.25)
        if stage < 6:
            return
        # ---- phase D/E: cross2 = cent2 @ E^T -> exp -> transpose -> softmax ----
        with (
            tc.tile_pool(name="crossps", bufs=1, space="PSUM") as cross_pool,
            tc.tile_pool(name="exps", bufs=NB) as exp_pool,
            tc.tile_pool(name="outtiles", bufs=4) as out_pool,
            tc.tile_pool(name="sums", bufs=8) as sum_pool,
        ):
            crs = [cross_pool.tile([P, 512], F32, name=f"cr{b}") for b in range(NB)]
            for j in range(DCH):
                for b in range(NB):
                    nc.tensor.matmul(
                        crs[b][:],
                        lhsT=centT[j][:],
                        rhs=et[:, j * NS + b * 512: j * NS + (b + 1) * 512],
                        start=(j == 0), stop=(j == DCH - 1),
                    )
            for b in range(NB):
                # exp(cross2 - sq_c) with per-partition bias; [C, 512] layout
                ex = exp_pool.tile([P, 512], F32, tag="exp")
                nc.scalar.activation(ex[:], crs[b][:], AF.Exp, bias=negsq[:, 0:1],
                                     scale=1.0)
                if stage < 7:
                    continue
                # back to [n, C] orientation in 128-col strips, then normalize rows
                for tt in range(4):
                    t = b * 4 + tt
                    tp2 = tr_ps.tile([P, P], F32, tag="tr")
                    nc.tensor.transpose(tp2[:], ex[:, tt * P:(tt + 1) * P], ident[:])
                    s = sum_pool.tile([P, 1], F32, tag="s")
                    nc.vector.reduce_sum(out=s[:], in_=tp2[:], axis=AX.X)
                    rs = sum_pool.tile([P, 1], F32, tag="rs")
                    nc.vector.reciprocal(rs[:], s[:])
                    ot = out_pool.tile([P, C], F32, tag="ot")
                    nc.scalar.activation(ot[:], tp2[:], AF.Copy, bias=0.0, scale=rs[:, 0:1])
                    nc.sync.dma_start(out=out[t * P:(t + 1) * P, :], in_=ot[:])


def build_module(stage: int = 7):
    nc = bacc.Bacc("TRN2", target_bir_lowering=False, debug=False,
                   num_devices=CORES)
    emb = nc.dram_tensor("embeddings", [NS, D], F32, kind="ExternalInput").ap()
    yt = nc.dram_tensor("y_true", [NS, C], F32, kind="ExternalInput").ap()
    out = nc.dram_tensor("out", [NS, C], F32, kind="ExternalOutput").ap()
    with tile.TileContext(nc) as tc:
        _build_kernel(tc, emb, yt, out, stage=stage)
    nc.compile()
    return nc


_NC_CACHE = {}


def _get_nc():
    if "nc" not in _NC_CACHE:
        _NC_CACHE["nc"] = build_module()
    return _NC_CACHE["nc"]


def run(embeddings: np.ndarray, y_true: np.ndarray, **spmd_kwargs):
    embeddings = np.ascontiguousarray(embeddings, dtype=np.float32)
    y_true = np.ascontiguousarray(y_true, dtype=np.float32)
    assert embeddings.shape == (N, D) and y_true.shape == (N, C)

    nc = _get_nc()
    in_maps = [
        {
            "embeddings": embeddings[k * NS:(k + 1) * NS],
            "y_true": y_true[k * NS:(k + 1) * NS],
        }
        for k in range(CORES)
    ]
    res = run_bass_kernel_spmd(nc, in_maps, core_ids=list(range(CORES)),
                               **spmd_kwargs)
    out = np.concatenate([res.results[k]["out"] for k in range(CORES)], axis=0)
    return out, res


def kernel(embeddings: np.ndarray, y_true: np.ndarray) -> np.ndarray:
    out, _ = run(embeddings, y_true)
    return out



# revision 6
# speedup vs baseline: 1.7286x; 1.7286x over previous
"""Centroid-similarity (ProtoNet softmax) kernel for 8 trn2 NeuronCores.

Math (per reference):
    counts   = sum_n y[n, c]
    cent     = (y^T @ E) / max(counts, 1)          # divide_no_nan
    out      = softmax(-(|e|^2 + |c|^2 - 2 e.c), axis=C)
softmax is invariant to per-row constants, so |e|^2 drops out:
    out      = softmax(2*cross - sq_c), cross = E @ cent^T, sq_c = |cent|^2

Distribution: data-parallel over N. Each core gets an N/8 = 2048-row shard,
computes partial (y^T E | counts) on the tensor engine, AllReduces the
[C, D+1] stats, then computes its own 2048 x C block of logits + softmax.

v2 layout/precision choices:
  - Inputs are converted to fp16 on the host and packed per row-chunk as
    [E_chunk | y_chunk] (one dram tensor, 2304B contiguous per partition
    row): halves HBM-in traffic vs fp32 and removes the f32r rounding
    copies entirely (fp16 matmul runs at full PE rate, 1 cycle/row).
  - The E shard is transposed on the PE (fp16: 1 cycle/row) while the
    AllReduce runs; mm2 contracts over D from the transposed copy.
  - The [C, D+1] stats AllReduce runs in fp16 (262KB instead of 525KB).
    Counts are exact in fp16 (integers < 2048); centroid-sum rounding adds
    ~1e-3 relative noise, well inside the 2e-2 gate.
  - exp() is written as bf16 (fp16 would overflow: logits reach e^22),
    which keeps the softmax-side transposes at 1 cycle/row.
"""

import numpy as np

import concourse.bacc as bacc
import concourse.bass as bass
import concourse.mybir as mybir
import concourse.tile as tile
from concourse import masks
from concourse.bass_utils import run_bass_kernel_spmd
from concourse.tile import add_dep_helper

N, C, D = 16384, 128, 1024
CORES = 8
NS = N // CORES          # 2048 rows per core
P = 128                  # partition dim
NCH = NS // P            # 16 n-chunks per core
DCH = D // P             # 8 d-chunks
NB = NS // 512           # 4 moving-dim blocks for matmul #2
W = D + C                # packed row: [e (1024) | y (128)] fp16

F32 = mybir.dt.float32
F16 = mybir.dt.float16
BF16 = mybir.dt.bfloat16
CC_DT = F16              # collective dtype; flip to F32 for exact stats

AF = mybir.ActivationFunctionType
AX = mybir.AxisListType


def _build_kernel(tc: tile.TileContext, pk: bass.AP, out: bass.AP,
                  stage: int = 7):
    nc = tc.nc

    with (
        tc.tile_pool(name="const", bufs=1) as const_pool,
        tc.tile_pool(name="persist", bufs=1) as persist,
        tc.tile_pool(name="chunks", bufs=NCH) as ch_pool,
        tc.tile_pool(name="dram", bufs=1, space="DRAM") as dram_pool,
        tc.tile_pool(name="trps", bufs=2, space="PSUM") as tr_ps,
        tc.tile_pool(name="smalls", bufs=1) as smalls,
    ):
        ident = const_pool.tile([P, P], F32)
        masks.make_identity(nc, ident[:])
        ident_h = const_pool.tile([P, P], F16)
        nc.scalar.copy(ident_h[:], ident[:])
        ident_b = const_pool.tile([P, P], BF16)
        nc.vector.tensor_copy(ident_b[:], ident[:])
        ones_h = const_pool.tile([P, 1], F16)
        nc.vector.memset(ones_h[:], 1.0)

        # ---- phase A: stream in packed fp16 shard; accumulate y^T E ----
        mm1_ctx = tc.tile_pool(name="mm1ps", bufs=1, space="PSUM")
        mm1_ps = mm1_ctx.__enter__()
        cent_ps = [mm1_ps.tile([P, 512], F32, name=f"cent_ps{h}") for h in range(2)]
        cnt_ps = mm1_ps.tile([P, 1], F32)
        pk_tiles = []
        for i in range(NCH):
            t = ch_pool.tile([P, W], F16, tag="pk")
            nc.sync.dma_start(out=t[:], in_=pk[i * P:(i + 1) * P, :])
            pk_tiles.append(t)
            if stage < 2:
                continue
            first, last = (i == 0), (i == NCH - 1)
            y_sl = t[:, D:D + C]
            for h in range(2):
                nc.tensor.matmul(
                    cent_ps[h][:],
                    lhsT=y_sl,
                    rhs=t[:, h * 512:(h + 1) * 512],
                    start=first, stop=last,
                )
            mm1_last = nc.tensor.matmul(
                cnt_ps[:], lhsT=y_sl, rhs=ones_h[:],
                start=first, stop=last,
            )

        if stage < 2:
            mm1_ctx.__exit__(None, None, None)
            return
        # ---- phase B: AllReduce the [C, D+1] stats across the 8 cores ----
        # high_priority: the stats -> AllReduce chain must win the per-engine
        # FIFO ordering races against the (long) phase-C transpose/copy work,
        # or the collective trigger slips down the ACT/gpsimd queues.
        stat_sb = persist.tile([P, D + 1], CC_DT)
        ar_in = dram_pool.tile([P, D + 1], CC_DT)
        ar_out = dram_pool.tile([P, D + 1], CC_DT)
        gcnt = persist.tile([P, 1], CC_DT)
        gcent = [persist.tile([P, P], CC_DT, name=f"gcent{j}") for j in range(DCH)]
        with tc.high_priority():
            nc.scalar.copy(stat_sb[:, 0:512], cent_ps[0][:])
            nc.scalar.copy(stat_sb[:, 512:1024], cent_ps[1][:])
            nc.scalar.copy(stat_sb[:, D:D + 1], cnt_ps[:])
            mm1_ctx.__exit__(None, None, None)
            if stage < 3:
                return
            ar_dma = nc.sync.dma_start(out=ar_in[:], in_=stat_sb[:])
            nc.gpsimd.collective_compute(
                "AllReduce",
                mybir.AluOpType.add,
                replica_groups=[list(range(CORES))],
                ins=[ar_in.opt()],
                outs=[ar_out.opt()],
            )
            # counts column first (tiny) so the reciprocal chain starts
            # before the centroid payload finishes landing
            nc.sync.dma_start(out=gcnt[:], in_=ar_out[:, D:D + 1])
            for j in range(DCH):
                nc.sync.dma_start(out=gcent[j][:],
                                  in_=ar_out[:, j * P:(j + 1) * P])

        if stage < 4:
            return
        # ---- phase C: transpose E shard on PE while the AllReduce runs ----
        et = persist.tile([P, DCH * NS], F16)  # d-chunk j at [:, j*NS:(j+1)*NS]
        k = 0
        for i in range(NCH):
            for j in range(DCH):
                tp = tr_ps.tile([P, P], F16, tag="tr")
                tr_inst = nc.tensor.transpose(
                    tp[:], pk_tiles[i][:, j * P:(j + 1) * P], ident_h[:])
                # ordering-only edge (same PE queue): keep all transposes
                # after phase A so they can't crowd out the stats ->
                # AllReduce chain
                add_dep_helper(tr_inst.ins, mm1_last.ins, sync=False,
                               reason="transposes after mm1")
                dst = et[:, j * NS + i * P: j * NS + (i + 1) * P]
                if k % 2 == 0:
                    cp_inst = nc.scalar.copy(dst, tp[:])
                else:
                    cp_inst = nc.vector.tensor_copy(dst, tp[:])
                # pin every ET copy after the AllReduce-input DMA so the
                # stats chain always wins the ACT/DVE FIFO placement races
                add_dep_helper(cp_inst.ins, ar_dma.ins, sync=False,
                               reason="et copies after ar_in dma")
                k += 1

        if stage < 5:
            return
        # ---- phase C2: cent2 = 2*cent, -sq_c, cent2^T — per d-chunk so
        # matmul #2's first accumulation starts as soon as chunk 0 is ready
        safe = smalls.tile([P, 1], F32)
        nc.vector.tensor_scalar_max(safe[:], gcnt[:], 1.0)
        r2 = smalls.tile([P, 1], F32)
        nc.vector.reciprocal(r2[:], safe[:])
        nc.vector.tensor_scalar_mul(r2[:], r2[:], 2.0)
        if stage < 5.4:
            return
        sq_tmp = persist.tile([P, D], F32)
        negsq = smalls.tile([P, 1], F32)
        cent2 = [persist.tile([P, P], F16, name=f"cent2_{j}") for j in range(DCH)]
        centT = [persist.tile([P, C], F16, name=f"centT{j}") for j in range(DCH)]
        for j in range(DCH):
            nc.vector.tensor_scalar_mul(cent2[j][:], gcent[j][:], r2[:, 0:1])
            tp = tr_ps.tile([P, P], F16, tag="tr")
            nc.tensor.transpose(tp[:], cent2[j][:], ident_h[:])
            nc.scalar.copy(centT[j][:], tp[:])
            # negsq contribution off the critical path (only exp needs it)
            nc.scalar.square(sq_tmp[:, j * P:(j + 1) * P], cent2[j][:])
        nc.vector.reduce_sum(out=negsq[:], in_=sq_tmp[:], axis=AX.X)
        nc.vector.tensor_scalar_mul(negsq[:], negsq[:], -0.25)
        if stage < 6:
            return
        # ---- phase D/E: cross2 = cent2 @ E^T -> exp -> transpose -> softmax
        # b-outer so block b's PSUM completes early and its softmax tail
        # overlaps block b+1's matmuls.
        with (
            tc.tile_pool(name="crossps", bufs=1, space="PSUM") as cross_pool,
            tc.tile_pool(name="tr2ps", bufs=2, space="PSUM") as tr2_ps,
            tc.tile_pool(name="exps", bufs=2) as exp_pool,
            tc.tile_pool(name="outtiles", bufs=4) as out_pool,
            tc.tile_pool(name="sums", bufs=8) as sum_pool,
        ):
            crs = [cross_pool.tile([P, 512], F32, name=f"cr{b}") for b in range(NB)]
            for b in range(NB):
                for j in range(DCH):
                    nc.tensor.matmul(
                        crs[b][:],
                        lhsT=centT[j][:],
                        rhs=et[:, j * NS + b * 512: j * NS + (b + 1) * 512],
                        start=(j == 0), stop=(j == DCH - 1),
                    )
                # exp(cross2 - sq_c) with per-partition bias; [C, 512] bf16
                ex = exp_pool.tile([P, 512], BF16, tag="exp")
                nc.scalar.activation(ex[:], crs[b][:], AF.Exp, bias=negsq[:, 0:1],
                                     scale=1.0)
                if stage < 7:
                    continue
                # back to [n, C] orientation in 128-col strips, then normalize
                for tt in range(4):
                    t = b * 4 + tt
                    tp2 = tr2_ps.tile([P, P], BF16, tag="tr2")
                    nc.tensor.transpose(tp2[:], ex[:, tt * P:(tt + 1) * P],
                                        ident_b[:])
                    s = sum_pool.tile([P, 1], F32, tag="s")
                    nc.vector.reduce_sum(out=s[:], in_=tp2[:], axis=AX.X)
                    rs = sum_pool.tile([P, 1], F32, tag="rs")
                    nc.vector.reciprocal(rs[:], s[:])
                    ot = out_pool.tile([P, C], F32, tag="ot")
                    nc.scalar.activation(ot[:], tp2[:], AF.Copy, bias=0.0,
                                         scale=rs[:, 0:1])
                    nc.sync.dma_start(out=out[t * P:(t + 1) * P, :], in_=ot[:])


def build_module(stage: int = 7):
    nc = bacc.Bacc("TRN2", target_bir_lowering=False, debug=False,
                   num_devices=CORES)
    pk = nc.dram_tensor("packed", [NS, W], F16, kind="ExternalInput").ap()
    out = nc.dram_tensor("out", [NS, C], F32, kind="ExternalOutput").ap()
    with tile.TileContext(nc) as tc:
        _build_kernel(tc, pk, out, stage=stage)
    nc.compile()
    return nc


_NC_CACHE = {}


def _get_nc():
    if "nc" not in _NC_CACHE:
        _NC_CACHE["nc"] = build_module()
    return _NC_CACHE["nc"]


def run(embeddings: np.ndarray, y_true: np.ndarray, **spmd_kwargs):
    assert embeddings.shape == (N, D) and y_true.shape == (N, C)
    emb16 = np.asarray(embeddings, dtype=np.float16)
    y16 = np.asarray(y_true, dtype=np.float16)
    packed = np.concatenate([emb16, y16], axis=1)  # [N, W] fp16

    nc = _get_nc()
    in_maps = [
        {"packed": np.ascontiguousarray(packed[k * NS:(k + 1) * NS])}
        for k in range(CORES)
    ]
    res = run_bass_kernel_spmd(nc, in_maps, core_ids=list(range(CORES)),
                               **spmd_kwargs)
    out = np.concatenate([res.results[k]["out"] for k in range(CORES)], axis=0)
    return out, res


def kernel(embeddings: np.ndarray, y_true: np.ndarray) -> np.ndarray:
    out, _ = run(embeddings, y_true)
    return out


# revision 9
# speedup vs baseline: 1.7331x; 1.0026x over previous
"""Centroid-similarity (ProtoNet softmax) kernel for 8 trn2 NeuronCores.

Math (per reference):
    counts   = sum_n y[n, c]
    cent     = (y^T @ E) / max(counts, 1)          # divide_no_nan
    out      = softmax(-(|e|^2 + |c|^2 - 2 e.c), axis=C)
softmax is invariant to per-row constants, so |e|^2 drops out:
    out      = softmax(2*cross - sq_c), cross = E @ cent^T, sq_c = |cent|^2

Distribution: data-parallel over N. Each core gets an N/8 = 2048-row shard,
computes partial (y^T E | counts) on the tensor engine, AllReduces the
[C, D+1] stats, then computes its own 2048 x C block of logits + softmax.

v2 layout/precision choices:
  - Inputs are converted to fp16 on the host and packed per row-chunk as
    [E_chunk | y_chunk] (one dram tensor, 2304B contiguous per partition
    row): halves HBM-in traffic vs fp32 and removes the f32r rounding
    copies entirely (fp16 matmul runs at full PE rate, 1 cycle/row).
  - The E shard is transposed on the PE (fp16: 1 cycle/row) while the
    AllReduce runs; mm2 contracts over D from the transposed copy.
  - The [C, D+1] stats AllReduce runs in fp16 (262KB instead of 525KB).
    Counts are exact in fp16 (integers < 2048); centroid-sum rounding adds
    ~1e-3 relative noise, well inside the 2e-2 gate.
  - exp() is written as bf16 (fp16 would overflow: logits reach e^22),
    which keeps the softmax-side transposes at 1 cycle/row.
"""

import numpy as np

import concourse.bacc as bacc
import concourse.bass as bass
import concourse.mybir as mybir
import concourse.tile as tile
from concourse import masks
from concourse.bass_utils import run_bass_kernel_spmd
from concourse.tile import add_dep_helper

N, C, D = 16384, 128, 1024
CORES = 8
NS = N // CORES          # 2048 rows per core
P = 128                  # partition dim
NCH = NS // P            # 16 n-chunks per core
DCH = D // P             # 8 d-chunks
NB = NS // 512           # 4 moving-dim blocks for matmul #2
W = D + C                # packed row: [e (1024) | y (128)] fp16

F32 = mybir.dt.float32
F16 = mybir.dt.float16
BF16 = mybir.dt.bfloat16
CC_DT = F32              # fp16 halves collective bytes but costs ~1e-2 rel err

AF = mybir.ActivationFunctionType
AX = mybir.AxisListType


def _build_kernel(tc: tile.TileContext, pk: bass.AP, out: bass.AP,
                  stage: int = 7):
    nc = tc.nc

    with (
        tc.tile_pool(name="const", bufs=1) as const_pool,
        tc.tile_pool(name="persist", bufs=1) as persist,
        tc.tile_pool(name="chunks", bufs=NCH) as ch_pool,
        tc.tile_pool(name="dram", bufs=1, space="DRAM") as dram_pool,
        tc.tile_pool(name="trps", bufs=2, space="PSUM") as tr_ps,
        tc.tile_pool(name="smalls", bufs=1) as smalls,
    ):
        ident = const_pool.tile([P, P], F32)
        masks.make_identity(nc, ident[:])
        ident_h = const_pool.tile([P, P], F16)
        nc.scalar.copy(ident_h[:], ident[:])
        ident_b = const_pool.tile([P, P], BF16)
        nc.vector.tensor_copy(ident_b[:], ident[:])
        ones_h = const_pool.tile([P, 1], F16)
        nc.vector.memset(ones_h[:], 1.0)

        # ---- phase A: stream in packed fp16 shard; accumulate y^T E ----
        mm1_ctx = tc.tile_pool(name="mm1ps", bufs=1, space="PSUM")
        mm1_ps = mm1_ctx.__enter__()
        cent_ps = [mm1_ps.tile([P, 512], F32, name=f"cent_ps{h}") for h in range(2)]
        cnt_ps = mm1_ps.tile([P, 1], F32)
        pk_tiles = []
        for i in range(NCH):
            t = ch_pool.tile([P, W], F16, tag="pk")
            nc.sync.dma_start(out=t[:], in_=pk[i * P:(i + 1) * P, :])
            pk_tiles.append(t)
            if stage < 2:
                continue
            first, last = (i == 0), (i == NCH - 1)
            y_sl = t[:, D:D + C]
            for h in range(2):
                nc.tensor.matmul(
                    cent_ps[h][:],
                    lhsT=y_sl,
                    rhs=t[:, h * 512:(h + 1) * 512],
                    start=first, stop=last,
                )
            mm1_last = nc.tensor.matmul(
                cnt_ps[:], lhsT=y_sl, rhs=ones_h[:],
                start=first, stop=last,
            )

        if stage < 2:
            mm1_ctx.__exit__(None, None, None)
            return
        # ---- phase B: AllReduce the [C, D+1] stats across the 8 cores ----
        # high_priority: the stats -> AllReduce chain must win the per-engine
        # FIFO ordering races against the (long) phase-C transpose/copy work,
        # or the collective trigger slips down the ACT/gpsimd queues.
        stat_sb = persist.tile([P, D + 1], CC_DT)
        ar_in = dram_pool.tile([P, D + 1], CC_DT)
        ar_out = dram_pool.tile([P, D + 1], CC_DT)
        gcnt = persist.tile([P, 1], CC_DT)
        gcent = [persist.tile([P, P], CC_DT, name=f"gcent{j}") for j in range(DCH)]
        with tc.high_priority():
            nc.scalar.copy(stat_sb[:, 0:512], cent_ps[0][:])
            nc.scalar.copy(stat_sb[:, 512:1024], cent_ps[1][:])
            nc.scalar.copy(stat_sb[:, D:D + 1], cnt_ps[:])
            mm1_ctx.__exit__(None, None, None)
            if stage < 3:
                return
            ar_dma = nc.sync.dma_start(out=ar_in[:], in_=stat_sb[:])
            nc.gpsimd.collective_compute(
                "AllReduce",
                mybir.AluOpType.add,
                replica_groups=[list(range(CORES))],
                ins=[ar_in.opt()],
                outs=[ar_out.opt()],
            )
            # counts column first (tiny) so the reciprocal chain starts
            # before the centroid payload finishes landing
            nc.sync.dma_start(out=gcnt[:], in_=ar_out[:, D:D + 1])
            for j in range(DCH):
                nc.sync.dma_start(out=gcent[j][:],
                                  in_=ar_out[:, j * P:(j + 1) * P])

        if stage < 4:
            return
        # ---- phase C: transpose E shard on PE while the AllReduce runs ----
        et = persist.tile([P, DCH * NS], F16)  # d-chunk j at [:, j*NS:(j+1)*NS]
        k = 0
        for i in range(NCH):
            for j in range(DCH):
                tp = tr_ps.tile([P, P], F16, tag="tr")
                tr_inst = nc.tensor.transpose(
                    tp[:], pk_tiles[i][:, j * P:(j + 1) * P], ident_h[:])
                # ordering-only edge (same PE queue): keep all transposes
                # after phase A so they can't crowd out the stats ->
                # AllReduce chain
                add_dep_helper(tr_inst.ins, mm1_last.ins, sync=False,
                               reason="transposes after mm1")
                dst = et[:, j * NS + i * P: j * NS + (i + 1) * P]
                if k % 2 == 0:
                    cp_inst = nc.scalar.copy(dst, tp[:])
                else:
                    cp_inst = nc.vector.tensor_copy(dst, tp[:])
                # pin every ET copy after the AllReduce-input DMA so the
                # stats chain always wins the ACT/DVE FIFO placement races
                add_dep_helper(cp_inst.ins, ar_dma.ins, sync=False,
                               reason="et copies after ar_in dma")
                k += 1

        if stage < 5:
            return
        # ---- phase C2: cent2 = 2*cent, -sq_c, cent2^T — per d-chunk so
        # matmul #2's first accumulation starts as soon as chunk 0 is ready
        safe = smalls.tile([P, 1], F32)
        nc.vector.tensor_scalar_max(safe[:], gcnt[:], 1.0)
        r2 = smalls.tile([P, 1], F32)
        nc.vector.reciprocal(r2[:], safe[:])
        nc.vector.tensor_scalar_mul(r2[:], r2[:], 2.0)
        if stage < 5.4:
            return
        sq_tmp = persist.tile([P, D], F32)
        negsq = smalls.tile([P, 1], F32)
        cent2 = [persist.tile([P, P], F16, name=f"cent2_{j}") for j in range(DCH)]
        centT = [persist.tile([P, C], F16, name=f"centT{j}") for j in range(DCH)]
        for j in range(DCH):
            nc.vector.tensor_scalar_mul(cent2[j][:], gcent[j][:], r2[:, 0:1])
            tp = tr_ps.tile([P, P], F16, tag="tr")
            nc.tensor.transpose(tp[:], cent2[j][:], ident_h[:])
            nc.scalar.copy(centT[j][:], tp[:])
            # negsq contribution off the critical path (only exp needs it)
            nc.scalar.square(sq_tmp[:, j * P:(j + 1) * P], cent2[j][:])
        nc.vector.reduce_sum(out=negsq[:], in_=sq_tmp[:], axis=AX.X)
        nc.vector.tensor_scalar_mul(negsq[:], negsq[:], -0.25)
        if stage < 6:
            return
        # ---- phase D/E: cross2 = cent2 @ E^T -> exp -> transpose -> softmax
        # b-outer so block b's PSUM completes early and its softmax tail
        # overlaps block b+1's matmuls.
        with (
            tc.tile_pool(name="crossps", bufs=1, space="PSUM") as cross_pool,
            tc.tile_pool(name="tr2ps", bufs=2, space="PSUM") as tr2_ps,
            tc.tile_pool(name="exps", bufs=2) as exp_pool,
            tc.tile_pool(name="outtiles", bufs=4) as out_pool,
            tc.tile_pool(name="sums", bufs=8) as sum_pool,
        ):
            crs = [cross_pool.tile([P, 512], F32, name=f"cr{b}") for b in range(NB)]
            for b in range(NB):
                for j in range(DCH):
                    nc.tensor.matmul(
                        crs[b][:],
                        lhsT=centT[j][:],
                        rhs=et[:, j * NS + b * 512: j * NS + (b + 1) * 512],
                        start=(j == 0), stop=(j == DCH - 1),
                    )
                # exp(cross2 - sq_c) with per-partition bias; [C, 512] fp32
                # (bf16/fp16 exp costs ~7e-3 rel err; fp16 would overflow)
                ex = exp_pool.tile([P, 512], F32, tag="exp")
                nc.scalar.activation(ex[:], crs[b][:], AF.Exp, bias=negsq[:, 0:1],
                                     scale=1.0)
                if stage < 7:
                    continue
                # back to [n, C] orientation in 128-col strips, then normalize
                for tt in range(4):
                    t = b * 4 + tt
                    tp2 = tr2_ps.tile([P, P], F32, tag="tr2")
                    nc.tensor.transpose(tp2[:], ex[:, tt * P:(tt + 1) * P],
                                        ident[:])
                    s = sum_pool.tile([P, 1], F32, tag="s")
                    nc.vector.reduce_sum(out=s[:], in_=tp2[:], axis=AX.X)
                    rs = sum_pool.tile([P, 1], F32, tag="rs")
                    nc.vector.reciprocal(rs[:], s[:])
                    ot = out_pool.tile([P, C], F32, tag="ot")
                    nc.scalar.activation(ot[:], tp2[:], AF.Copy, bias=0.0,
                                         scale=rs[:, 0:1])
                    nc.sync.dma_start(out=out[t * P:(t + 1) * P, :], in_=ot[:])


def build_module(stage: int = 7):
    nc = bacc.Bacc("TRN2", target_bir_lowering=False, debug=False,
                   num_devices=CORES)
    pk = nc.dram_tensor("packed", [NS, W], F16, kind="ExternalInput").ap()
    out = nc.dram_tensor("out", [NS, C], F32, kind="ExternalOutput").ap()
    with tile.TileContext(nc) as tc:
        _build_kernel(tc, pk, out, stage=stage)
    nc.compile()
    return nc


_NC_CACHE = {}


def _get_nc():
    if "nc" not in _NC_CACHE:
        _NC_CACHE["nc"] = build_module()
    return _NC_CACHE["nc"]


def run(embeddings: np.ndarray, y_true: np.ndarray, **spmd_kwargs):
    assert embeddings.shape == (N, D) and y_true.shape == (N, C)
    emb16 = np.asarray(embeddings, dtype=np.float16)
    y16 = np.asarray(y_true, dtype=np.float16)
    packed = np.concatenate([emb16, y16], axis=1)  # [N, W] fp16

    nc = _get_nc()
    in_maps = [
        {"packed": np.ascontiguousarray(packed[k * NS:(k + 1) * NS])}
        for k in range(CORES)
    ]
    res = run_bass_kernel_spmd(nc, in_maps, core_ids=list(range(CORES)),
                               **spmd_kwargs)
    out = np.concatenate([res.results[k]["out"] for k in range(CORES)], axis=0)
    return out, res


def kernel(embeddings: np.ndarray, y_true: np.ndarray) -> np.ndarray:
    out, _ = run(embeddings, y_true)
    return out
